# revision 2
# baseline (speedup 1.0000x reference)
"""DeepseekV2-Lite MLA-vanilla attention block on 8 Trainium2 NeuronCores, v3.

Sharding: tensor-parallel over the 16 heads (4 groups of 4 heads) x
data-parallel over batch (2) -> 8 cores; host sums the 4 partials per batch.

v3 (vs v2's 411us):
  - ckv-before-q per nt tile, with the SAME tile's norm / kv_b popped under
    the q-projection stream: nt3's RMSNorm Sqrt retires ~10us before the
    first attention Exp, so the ACT table is loaded exactly twice
    (sqrt set, then exp set) instead of thrashing mid-attention.
  - h8/hb DMA for tile nt+1 prefetched during tile nt's q-projection
    (the v2 loads had zero lead time -> 3.4us PE gap per nt boundary).
  - Attention exp runs on SINGLE score tiles with psS bufs=4: v2's
    [128,2,512] pair exps + bufs=2 serialized PE->ACT->PE at 1.74us per
    pair; singles give the chain 4 tiles of slack so the phase runs at
    max(PE, ACT) instead of PE+ACT. Diagonal tiles trim scores/exp/mask/
    AV/rowsum to the causal range.
  - PSUM evacuations: ck/kn/v ride ACT (idle during projections), q stays
    on DVE; the Wo drain alternates ACT/DVE and runs 6 output buffers deep.
"""

import sys

sys.path.insert(0, "/opt/trn_rl_repo")

import numpy as np
import ml_dtypes

import concourse.bass as bass  # noqa: F401
import concourse.mybir as mybir
import concourse.tile as tile
from concourse import bacc
from concourse.bass_utils import run_bass_kernel_spmd

B, S, HID = 2, 2048, 2048
NH, D_NOPE, D_ROPE, D_Q, D_V, LORA = 16, 128, 64, 192, 128, 512
SCALE = D_Q ** -0.5
EPS = 1e-6
G = 4          # head groups (tensor parallel)
HPG = NH // G  # heads per group
N_CORES = 8
NT = S // 512  # 512-token tiles
TT = S // 128  # 128-token tiles
WSCALE = 64.0              # host-side fp8 weight scale
QS = 8.0 * SCALE / WSCALE  # psum -> qT fp8 scale (q stored as 8*SCALE*q)

TRACE = False

f32 = mybir.dt.float32
f32r = mybir.dt.float32r
bf16 = mybir.dt.bfloat16
f8e4 = mybir.dt.float8e4
DR = mybir.MatmulPerfMode.DoubleRow

_compiled = None


def _build():
    FT = mybir.ActivationFunctionType
    OP = mybir.AluOpType

    nc = bacc.Bacc("TRN2", target_bir_lowering=False, debug=False,
                   num_devices=N_CORES)

    hq8 = nc.dram_tensor("hq8", [HID, S], f8e4, kind="ExternalInput").ap()
    hqb = nc.dram_tensor("hqb", [HID, S], bf16, kind="ExternalInput").ap()
    wq8 = nc.dram_tensor("wq8", [HID, HPG * D_Q], f8e4, kind="ExternalInput").ap()
    wkvaT = nc.dram_tensor("wkvaT", [HID, LORA + D_ROPE], bf16, kind="ExternalInput").ap()
    wkvbkT = nc.dram_tensor("wkvbkT", [LORA, HPG * D_NOPE], bf16, kind="ExternalInput").ap()
    wkvbvT = nc.dram_tensor("wkvbvT", [LORA, HPG * D_V], bf16, kind="ExternalInput").ap()
    woT = nc.dram_tensor("woT", [HPG * D_V, HID], bf16, kind="ExternalInput").ap()
    cs = nc.dram_tensor("cs", [128, 2], f32, kind="ExternalInput").ap()
    masks = nc.dram_tensor("masks", [128, 4, 512], bf16, kind="ExternalInput").ap()
    onecb = nc.dram_tensor("onecb", [128, 1], bf16, kind="ExternalInput").ap()
    outp = nc.dram_tensor("outp", [S, HID], f32, kind="ExternalOutput").ap()

    hq8_r = hq8.rearrange("(kp two p) t -> p kp two t", p=128, two=2)  # [128,8,2,S]
    hqb_r = hqb.rearrange("(ko p) t -> p ko t", p=128)                 # [128,16,S]
    wq8_r = wq8.rearrange("(kp two p) f -> p kp two f", p=128, two=2)  # [128,8,2,768]
    wkvaT_r = wkvaT.rearrange("(ko p) f -> p ko f", p=128)             # [128,16,576]
    wkvbkT_r = wkvbkT.rearrange("(c p) f -> p c f", p=128)             # [128,4,512]
    wkvbvT_r = wkvbvT.rearrange("(c p) f -> p c f", p=128)             # [128,4,512]
    woT_r = woT.rearrange("(c p) o -> p c o", p=128)                   # [128,4,HID]

    with tile.TileContext(nc) as tc, nc.allow_low_precision(
        reason="bf16/fp8 rounding of matmul operands is the design"
    ):
        with (
            tc.tile_pool(name="const", bufs=1, side="right") as const,
            tc.tile_pool(name="aop", bufs=1, side="right") as aop,
            tc.tile_pool(name="wop", bufs=1, side="right") as wop,
        ):
            c_onecb = const.tile([128, 1], bf16)
            nc.sync.dma_start(c_onecb[:], onecb)
            c_cs = const.tile([128, 2], f32)
            nc.sync.dma_start(c_cs[:], cs)
            c_eps = const.tile([1, 1], f32)
            nc.gpsimd.memset(c_eps[:], EPS)
            wscr = const.tile([128, 512], bf16)
            nc.gpsimd.memset(wscr[:], 0.25)

            ao_sb = aop.tile([128, HPG, S], bf16, tag="ao")   # [d_v, head, t]
            wo_sb = wop.tile([128, 4, HID], bf16, tag="wo")

            with tc.tile_pool(name="keep", bufs=1, side="right") as keep:
                # qT: [:, h, 0, :] = fp8(8*SCALE*q_nope_h)
                #     [:, h, 1, :] = fp8(8*SCALE*q_pe_h) on parts 0:64, 0 above
                qT = keep.tile([128, HPG, 2, S], f8e4, tag="qT")
                # kk: [:, h, 0, :] = fp8(k_nope_h); [:, h, 1, :] = fp8(k_pe)
                # (replicated per head; parts 64:128 zero)
                kk = keep.tile([128, HPG, 2, S], f8e4, tag="kk")
                v_sb = keep.tile([128, TT, HPG * D_V], bf16, tag="v")

                # ---------------- projection phase (fused B1+A) -------------
                with (
                    tc.tile_pool(name="wqp", bufs=1) as wqp,
                    tc.tile_pool(name="wkp", bufs=1) as wkp,
                    tc.tile_pool(name="hs8p", bufs=2) as hs8p,
                    tc.tile_pool(name="hsbp", bufs=2) as hsbp,
                    tc.tile_pool(name="ckp", bufs=2) as ckp,
                    tc.tile_pool(name="ntp", bufs=2) as ntp,
                    tc.tile_pool(name="psW", bufs=1, space="PSUM") as psW,
                    tc.tile_pool(name="psP", bufs=2, space="PSUM") as psP,
                    tc.tile_pool(name="psB", bufs=2, space="PSUM") as psB,
                    tc.tile_pool(name="psN", bufs=1, space="PSUM") as psN,
                ):
                    # --- DMA priority order: what the first matmuls need ---
                    wkva_sb = wkp.tile([128, 16, 576], bf16, tag="wkva")
                    for k in range(16):
                        nc.sync.dma_start(wkva_sb[:, k], wkvaT_r[:, k])

                    def load_h8(nt):
                        h8 = hs8p.tile([128, 8, 2, 512], f8e4, tag="h8",
                                       name="h8")
                        nts = slice(nt * 512, (nt + 1) * 512)
                        for kp in range(8):
                            nc.sync.dma_start(h8[:, kp], hq8_r[:, kp, :, nts])
                        return h8

                    def load_hb(nt):
                        hb = hsbp.tile([128, 16, 512], bf16, tag="hb",
                                       name="hb")
                        nts = slice(nt * 512, (nt + 1) * 512)
                        for k in range(16):
                            nc.sync.dma_start(hb[:, k], hqb_r[:, k, nts])
                        return hb

                    hb = load_hb(0)

                    # HAM warmup: keep the PE busy from t=0 until real work.
                    pw = psW.tile([128, 512], f32, tag="pw")
                    for i in range(30):
                        nc.tensor.matmul(pw[:], wscr[:, 0:128], wscr[:],
                                         start=(i == 0), stop=(i == 29))

                    wq_sb = wqp.tile([128, 8, 2, HPG * D_Q], f8e4)
                    for kp in range(8):
                        nc.sync.dma_start(wq_sb[:, kp], wq8_r[:, kp])
                    h8 = load_h8(0)
                    wbk = wkp.tile([128, 4, 512], bf16, tag="wbk")
                    wbv = wkp.tile([128, 4, 512], bf16, tag="wbv")
                    for c in range(4):
                        nc.sync.dma_start(wbk[:, c], wkvbkT_r[:, c])
                        nc.sync.dma_start(wbv[:, c], wkvbvT_r[:, c])

                    # zero strips: q_pe and k_pe pads above partition 64
                    for h in range(HPG):
                        nc.gpsimd.memset(qT[64:128, h, 1, :], 0)
                        nc.gpsimd.memset(kk[64:128, h, 1, :], 0)

                    def rope64(dst64, src64):
                        # dst = src*cos + rot_half(src)*s2 on 64 partitions
                        rq = ntp.tile([64, 512], bf16, tag="rq", name="rq")
                        nc.vector.tensor_copy(rq[0:32], src64[32:64])
                        nc.vector.tensor_copy(rq[32:64], src64[0:32])
                        nc.vector.tensor_scalar_mul(dst64, src64,
                                                    c_cs[0:64, 0:1])
                        nc.vector.tensor_scalar_mul(rq[:], rq[:],
                                                    c_cs[0:64, 1:2])
                        nc.vector.tensor_add(dst64, dst64, rq[:])

                    def norm_rope_nt(ck, nt):
                        # RMS-normalize ck chunks 0..3 in place, RoPE chunk 4
                        # into kk[:, :, 1] (replicated per head, fp8).
                        nts = slice(nt * 512, (nt + 1) * 512)
                        ssq = psN.tile([1, 512], f32, tag="ssq", name="ssq")
                        for c in range(4):
                            sq = ntp.tile([128, 512], bf16, tag="sq", name="sq")
                            nc.vector.tensor_tensor(sq[:], ck[:, c, :],
                                                    ck[:, c, :], OP.mult)
                            nc.tensor.matmul(ssq[:], c_onecb[:], sq[:],
                                             start=(c == 0), stop=(c == 3))
                        rms = ntp.tile([1, 512], f32, tag="rms", name="rms")
                        nc.scalar.activation(rms[:], ssq[:], FT.Sqrt,
                                             scale=1.0 / LORA, bias=c_eps[:])
                        bcs = ntp.tile([128, 512], f32, tag="bcs", name="bcs")
                        nc.gpsimd.partition_broadcast(bcs[:], rms[:], 128)
                        rbc = ntp.tile([128, 512], f32, tag="rbc", name="rbc")
                        nc.vector.reciprocal_approx_fast(rbc[:], bcs[:])
                        for c in range(4):
                            nc.vector.tensor_tensor(ck[:, c, :], ck[:, c, :],
                                                    rbc[:], OP.mult)
                        kpe = ntp.tile([64, 512], bf16, tag="kpe", name="kpe")
                        rope64(kpe[:], ck[0:64, 4, :])
                        for h in range(HPG):
                            nc.vector.tensor_copy(kk[0:64, h, 1, nts], kpe[:])

                    def kvb_kn_nt(ck, nt):
                        nts = slice(nt * 512, (nt + 1) * 512)
                        for m in range(HPG):
                            pm = psB.tile([128, 512], f32, tag="pb", name="pm")
                            for c in range(4):
                                nc.tensor.matmul(
                                    pm[:], wbk[:, c, m * 128:(m + 1) * 128],
                                    ck[:, c, :], start=(c == 0), stop=(c == 3))
                            nc.scalar.copy(kk[:, m, 0, nts], pm[:])

                    def kvb_v_nt(ck, nt):
                        for ti in range(4):
                            tt = nt * 4 + ti
                            pv = psB.tile([128, 512], f32, tag="pb", name="pv")
                            for c in range(4):
                                nc.tensor.matmul(
                                    pv[:], ck[:, c, ti * 128:(ti + 1) * 128],
                                    wbv[:, c, :], start=(c == 0), stop=(c == 3))
                            nc.scalar.copy(v_sb[:, tt, :], pv[:])

                    M_CKV = ((0, 128), (128, 128), (256, 128), (384, 128),
                             (512, 64))
                    for nt in range(NT):
                        nts = slice(nt * 512, (nt + 1) * 512)
                        # ckv projection: bf16, m-outer k-inner
                        ck = ckp.tile([128, 5, 512], bf16, tag="ck", name="ck")
                        for m, (mo, mw) in enumerate(M_CKV):
                            pa = psP.tile([128, 512], f32, tag="pp", name="pa")
                            for k in range(16):
                                nc.tensor.matmul(
                                    pa[:mw], wkva_sb[:, k, mo:mo + mw],
                                    hb[:, k], start=(k == 0), stop=(k == 15))
                            nc.scalar.copy(ck[:mw, m, :], pa[:mw])
                        # deferred into this nt's q stream: norm + kv_b
                        deferred = [
                            (lambda ck=ck, nt=nt: norm_rope_nt(ck, nt)),
                            (lambda ck=ck, nt=nt: kvb_kn_nt(ck, nt)),
                            (lambda ck=ck, nt=nt: kvb_v_nt(ck, nt)),
                        ]
                        if nt + 1 < NT:
                            hb = load_hb(nt + 1)
                        # q projection: DoubleRow fp8, m-outer k-inner
                        for m in range(6):
                            pq = psP.tile([128, 512], f32, tag="pp", name="pq")
                            for kp in range(8):
                                nc.tensor.matmul(
                                    pq[:],
                                    wq_sb[:, kp, :, m * 128:(m + 1) * 128],
                                    h8[:, kp], start=(kp == 0), stop=(kp == 7),
                                    perf_mode=DR)
                            if m < 4:
                                nc.vector.tensor_scalar_mul(
                                    qT[:, m, 0, nts], pq[:], QS)
                            else:
                                for i in range(2):
                                    h = (m - 4) * 2 + i
                                    nc.vector.tensor_scalar_mul(
                                        qT[0:64, h, 1, nts],
                                        pq[i * 64:(i + 1) * 64], QS)
                            if m in (1, 3, 5):
                                deferred.pop(0)()
                            if m == 0 and nt + 1 < NT:
                                h8_next = load_h8(nt + 1)
                        if nt + 1 < NT:
                            h8 = h8_next
                        # rope on q_pe (fp8, in place)
                        for h in range(HPG):
                            rope64(qT[0:64, h, 1, nts],
                                   qT[0:64, h, 1, nts])

                # ---------------- attention --------------------------------
                with (
                    tc.tile_pool(name="mkp", bufs=1) as mkp,
                    tc.tile_pool(name="pTp", bufs=3) as pTp,
                    tc.tile_pool(name="smp", bufs=2) as smp,
                    tc.tile_pool(name="psS", bufs=4, space="PSUM") as psS,
                    tc.tile_pool(name="psAV", bufs=2, space="PSUM") as psAV,
                    tc.tile_pool(name="psSE", bufs=2, space="PSUM") as psSE,
                ):
                    for c in range(4):
                        nc.sync.dma_start(wo_sb[:, c], woT_r[:, c])
                    c_masks = mkp.tile([128, 4, 512], bf16)
                    nc.sync.dma_start(c_masks[:], masks)

                    def fin(se, av, h, its):
                        # rowsums -> broadcast -> reciprocal -> normalized ao
                        # write. No PE work.
                        se_sb = smp.tile([1, 512], f32, tag="ses", name="ses")
                        nc.vector.tensor_copy(se_sb[:], se[:])
                        seb = smp.tile([128, 512], f32, tag="seb", name="seb")
                        nc.gpsimd.partition_broadcast(seb[:], se_sb[:], 128)
                        rbc = smp.tile([128, 512], f32, tag="rb2", name="rb2")
                        nc.vector.reciprocal_approx_fast(rbc[:], seb[:])
                        nc.vector.tensor_tensor(ao_sb[:, h, its], av[:],
                                                rbc[:], OP.mult)

                    slots = [(it, h) for it in (2, 3, 1, 0) for h in range(HPG)]
                    sched = [[slots[0], slots[1]]] + [[s] for s in slots[2:]] \
                        + [[], []]
                    pending = []
                    fins = []
                    for group in sched:
                        news = []
                        for it_c, h_c in group:
                            njt_c = 4 * it_c + 4
                            pT = pTp.tile([128, TT, 512], bf16, tag="pT",
                                          name="pT")
                            news.append((pT, it_c, h_c, njt_c))
                        if pending:
                            pT_p, it_p, h_p, njt_p = pending.pop(0)
                            its_p = slice(it_p * 512, (it_p + 1) * 512)
                            se = psSE.tile([1, 512], f32, tag="se", name="se")
                            av = psAV.tile([128, 512], f32, tag="av", name="av")
                        else:
                            njt_p = 0
                        jt_max = max([njt_p] + [n[3] for n in news])
                        for jt in range(jt_max):
                            jts = slice(jt * 128, (jt + 1) * 128)
                            for pT, it_c, h_c, njt_c in news:
                                if jt >= njt_c:
                                    continue
                                # causal trim: diagonal tiles only need
                                # queries i >= kd*128 within the it-tile
                                t0 = max(0, jt - 4 * it_c) * 128
                                iss = slice(it_c * 512 + t0, (it_c + 1) * 512)
                                st = psS.tile([128, 512], f32, tag="sT",
                                              name="sT")
                                nc.tensor.matmul(
                                    st[:, t0:], kk[:, h_c, :, jts],
                                    qT[:, h_c, :, iss],
                                    start=True, stop=True, perf_mode=DR)
                                nc.scalar.activation(
                                    pT[:, jt, t0:], st[:, t0:], FT.Exp,
                                    scale=0.125)
                                kd = jt - 4 * it_c
                                if kd >= 0:
                                    nc.vector.tensor_tensor(
                                        pT[:, jt, t0:], pT[:, jt, t0:],
                                        c_masks[:, kd, t0:], OP.mult)
                            if jt == 1 and fins:
                                fins.pop(0)()
                            if njt_p and jt < njt_p:
                                t0 = max(0, jt - 4 * it_p) * 128
                                nc.tensor.matmul(
                                    se[0:1, t0:], c_onecb[:],
                                    pT_p[:, jt, t0:],
                                    start=(jt == 0), stop=(jt == njt_p - 1))
                                nc.tensor.matmul(
                                    av[:, t0:],
                                    v_sb[:, jt, h_p * 128:(h_p + 1) * 128],
                                    pT_p[:, jt, t0:],
                                    start=(jt == 0), stop=(jt == njt_p - 1))
                        if njt_p:
                            fins.append(lambda se=se, av=av, h=h_p,
                                        its=its_p: fin(se, av, h, its))
                        pending.extend(news)
                    while fins:
                        fins.pop(0)()

            # ---------------- Wo projection --------------------------------
            with (
                tc.tile_pool(name="outs", bufs=6) as osp,
                tc.tile_pool(name="psO", bufs=4, space="PSUM") as psO,
            ):
                n_out = 0
                for tt in [8, 9, 10, 11, 12, 13, 14, 15, 4, 5, 6, 7,
                           0, 1, 2, 3]:
                    tts = slice(tt * 128, (tt + 1) * 128)
                    for ot in range(4):
                        ots = slice(ot * 512, (ot + 1) * 512)
                        po = psO.tile([128, 512], f32, tag="po", name="po")
                        for c in range(4):
                            nc.tensor.matmul(po[:], ao_sb[:, c, tts],
                                             wo_sb[:, c, ots],
                                             start=(c == 0), stop=(c == 3))
                        ob = osp.tile([128, 512], f32, tag="ob", name="ob")
                        if n_out % 2 == 0:
                            nc.scalar.copy(ob[:], po[:])
                        else:
                            nc.vector.tensor_copy(ob[:], po[:])
                        n_out += 1
                        nc.sync.dma_start(outp[tts, ots], ob[:])

    nc.compile()
    return nc


def _get_compiled():
    global _compiled
    if _compiled is None:
        _compiled = _build()
    return _compiled


def _f8(x):
    return np.clip(x, -240, 240).astype(ml_dtypes.float8_e4m3fn)


def _host_prep(hidden_states, Wq, Wkva, kv_a_norm_weight, Wkvb, Wo, cos, sin):
    hs = np.asarray(hidden_states, dtype=np.float32)
    Wq = np.asarray(Wq, dtype=np.float32)
    Wkva = np.asarray(Wkva, dtype=np.float32)
    w_norm = np.asarray(kv_a_norm_weight, dtype=np.float32)
    Wkvb = np.asarray(Wkvb, dtype=np.float32) * w_norm[None, :]
    Wo = np.asarray(Wo, dtype=np.float32)
    cos64 = np.asarray(cos, dtype=np.float32).reshape(D_ROPE)
    sin64 = np.asarray(sin, dtype=np.float32).reshape(D_ROPE)

    wkvaT = np.ascontiguousarray(Wkva.T).astype(ml_dtypes.bfloat16)  # [HID, 576]
    s2 = np.concatenate([-sin64[:32], sin64[32:]])
    cs_host = np.ascontiguousarray(
        np.stack([np.tile(cos64, 2), np.tile(s2, 2)], axis=1))  # [128, 2]
    jj = np.arange(128)[:, None, None]
    kd = np.arange(4)[None, :, None]
    ii = np.arange(512)[None, None, :]
    masks_host = (kd * 128 + jj <= ii).astype(ml_dtypes.bfloat16)  # [128,4,512]
    onecb_h = np.ones((128, 1), dtype=ml_dtypes.bfloat16)

    hsTs = [np.ascontiguousarray(hs[b].T) for b in range(B)]
    hq8s = [_f8(t) for t in hsTs]
    hqbs = [t.astype(ml_dtypes.bfloat16) for t in hsTs]

    in_maps = []
    for core in range(N_CORES):
        b, g = divmod(core, G)
        heads = list(range(g * HPG, (g + 1) * HPG))
        wq_rows = np.concatenate(
            [Wq[h * D_Q:h * D_Q + D_NOPE] for h in heads]
            + [Wq[h * D_Q + D_NOPE:(h + 1) * D_Q] for h in heads], axis=0)
        wq8_h = _f8(np.ascontiguousarray(wq_rows.T) * WSCALE)      # [HID, 768]
        wkvbkT_h = np.ascontiguousarray(np.concatenate(
            [Wkvb[h * 256:h * 256 + 128] for h in heads],
            axis=0).T).astype(ml_dtypes.bfloat16)                  # [LORA, 512]
        wkvbvT_h = np.ascontiguousarray(np.concatenate(
            [Wkvb[h * 256 + 128:h * 256 + 256] for h in heads],
            axis=0).T).astype(ml_dtypes.bfloat16)
        woT_h = np.ascontiguousarray(np.concatenate(
            [Wo[:, h * D_V:(h + 1) * D_V] for h in heads],
            axis=1).T).astype(ml_dtypes.bfloat16)                  # [512, HID]
        in_maps.append({
            "hq8": hq8s[b], "hqb": hqbs[b], "wq8": wq8_h, "wkvaT": wkvaT,
            "wkvbkT": wkvbkT_h, "wkvbvT": wkvbvT_h, "woT": woT_h,
            "cs": cs_host, "masks": masks_host, "onecb": onecb_h,
        })
    return in_maps


def _install_ntff_hook():
    """Register the axon NTFF profiling hook (missing antenv.axon_hooks stub)."""
    import types

    if "antenv.axon_hooks" in sys.modules:
        return
    import antenv  # noqa: F401
    mod = types.ModuleType("antenv.axon_hooks")
    mod._hook = None
    mod.set_axon_ntff_profile_hook = lambda h: setattr(mod, "_hook", h)
    mod.get_axon_ntff_profile_hook = lambda: mod._hook
    sys.modules["antenv.axon_hooks"] = mod
    try:
        from trn_agent_boot.trn_boot import _ntff_profile_via_ctypes
        mod._hook = _ntff_profile_via_ctypes("/opt/axon/libaxon_pjrt.so")
    except Exception as e:  # profiling is best-effort
        print(f"ntff hook install failed: {e}")


def kernel(hidden_states, Wq, Wkva, kv_a_norm_weight, Wkvb, Wo, cos, sin):
    in_maps = _host_prep(hidden_states, Wq, Wkva, kv_a_norm_weight,
                         Wkvb, Wo, cos, sin)
    if TRACE:
        _install_ntff_hook()
    nc = _get_compiled()
    res = run_bass_kernel_spmd(nc, in_maps, core_ids=list(range(N_CORES)),
                               trace=TRACE)
    kernel.last_result = res
    out = np.zeros((B, S, HID), dtype=np.float32)
    for core in range(N_CORES):
        b = core // G
        out[b] += res.results[core]["outp"]
    return out
